# revision 4
# baseline (speedup 1.0000x reference)
"""Embedding lookup (nn.Embedding forward) on 8 TRN2 NeuronCores.

Strategy (per the row-sharding hint, with the index routing done host-side):
the 1M x 128 fp32 table is row-sharded into 8 contiguous shards of 131072
rows (table padded to 1,048,576 rows), one per core -- 64 MB each.  The host
routes each of the 2,097,152 indices to the owning core, and within a core to
one of four 32768-row windows, so the on-device gather can use the bulk
`dma_gather` instruction (int16 local indices, one 512 B descriptor per row,
descriptor generation spread across the 8 GpSimd Q7 cores).  Each (core,
window) bucket is padded to a fixed capacity so all 8 cores run the same SPMD
program; the host applies the inverse permutation to the concatenated per-core
outputs to restore the original index order.

Per-core HW traffic: ~147 MB gather reads + ~147 MB output writes at the
~360 GB/s per-core HBM limit -> ~820 us roofline.
"""

import sys

if "/opt/trn_rl_repo" not in sys.path:
    sys.path.insert(0, "/opt/trn_rl_repo")

import numpy as np

N_CORES = 8
N_EMB = 1_000_000
D = 128
N_IDX = 2_097_152
P = 128

WINDOW = 32768                     # rows addressable by one int16 gather
BUCKETS_PER_CORE = 4
SHARD_ROWS = WINDOW * BUCKETS_PER_CORE      # 131072
N_EMB_PAD = SHARD_ROWS * N_CORES            # 1048576
N_BUCKETS = N_CORES * BUCKETS_PER_CORE      # 32

CHUNK_IDX = 14336                  # indices per dma_gather (nblk = 112)
NBLK = CHUNK_IDX // P              # 112
CHUNKS = 5                         # chunks per bucket
CAP = CHUNK_IDX * CHUNKS           # 71680 padded capacity per bucket
N_GATHERS = BUCKETS_PER_CORE * CHUNKS       # 20 per core
OUT_PER_CORE = CAP * BUCKETS_PER_CORE       # 286720 rows
IDX_COLS = CHUNK_IDX // 16         # 896 int16 per partition per chunk

_NC_CACHE = None


def _build_nc():
    global _NC_CACHE
    if _NC_CACHE is not None:
        return _NC_CACHE

    from concourse import bacc, mybir, tile

    nc = bacc.Bacc("TRN2", target_bir_lowering=False, debug=False,
                   num_devices=N_CORES)
    w = nc.dram_tensor("wshard", (SHARD_ROWS, D), mybir.dt.float32,
                       kind="ExternalInput")
    idxt = nc.dram_tensor("idx", (N_GATHERS, P, IDX_COLS), mybir.dt.int16,
                          kind="ExternalInput")
    out = nc.dram_tensor("out", (OUT_PER_CORE, D), mybir.dt.float32,
                         kind="ExternalOutput")

    with tile.TileContext(nc) as tc:
        with tc.tile_pool(name="ip", bufs=2) as ip, \
             tc.tile_pool(name="gp", bufs=2) as gp:
            for b in range(BUCKETS_PER_CORE):
                win = w[b * WINDOW:(b + 1) * WINDOW, :]
                for t in range(CHUNKS):
                    k = b * CHUNKS + t
                    it = ip.tile([P, IDX_COLS], mybir.dt.int16)
                    nc.sync.dma_start(it[:], idxt[k, :, :])
                    g = gp.tile([P, NBLK * D], mybir.dt.float32)
                    nc.gpsimd.dma_gather(
                        out_ap=g[:].rearrange("p (n d) -> p n d", d=D),
                        in_ap=win,
                        idxs_ap=it[:],
                        num_idxs=CHUNK_IDX,
                        num_idxs_reg=CHUNK_IDX,
                        elem_size=D,
                        single_packet=False,
                    )
                    # DRAM row k*CHUNK_IDX + p*NBLK + j  <-  tile[p, j]
                    dst = out[k * CHUNK_IDX:(k + 1) * CHUNK_IDX, :]
                    nc.sync.dma_start(
                        dst.rearrange("(p n) d -> p n d", p=P), g[:]
                    )

    nc.compile()
    _NC_CACHE = nc
    return nc


def _ensure_ntff_hook():
    """The agent image's antenv lacks axon_hooks, so run_bass_kernel_spmd's
    trace path can't find the NTFF profile hook trn_boot builds.  Shim the
    module and install the ctypes hook ourselves; also neuter the bucket
    upload (no artifact store in this container)."""
    import sys as _sys
    import types

    if "antenv.axon_hooks" not in _sys.modules:
        mod = types.ModuleType("antenv.axon_hooks")
        mod._hook = None

        def set_axon_ntff_profile_hook(h):
            mod._hook = h

        def get_axon_ntff_profile_hook():
            return mod._hook

        mod.set_axon_ntff_profile_hook = set_axon_ntff_profile_hook
        mod.get_axon_ntff_profile_hook = get_axon_ntff_profile_hook
        _sys.modules["antenv.axon_hooks"] = mod
        import antenv

        antenv.axon_hooks = mod

    from antenv.axon_hooks import (get_axon_ntff_profile_hook,
                                   set_axon_ntff_profile_hook)

    if get_axon_ntff_profile_hook() is None:
        from trn_agent_boot.trn_boot import _ntff_profile_via_ctypes

        set_axon_ntff_profile_hook(
            _ntff_profile_via_ctypes("/opt/axon/libaxon_pjrt.so")
        )

    from concourse import bass_utils

    bass_utils.upload_artifacts = lambda tmpdir: f"local://{tmpdir}"


def _route(index):
    """Host-side routing: bucket each index by value, pad buckets to CAP,
    build the per-core int16 gather-index tiles and the gather->original
    permutation."""
    idx64 = np.asarray(index).astype(np.int64)
    g = idx64 >> 15                                  # owning bucket, 0..30
    order = np.argsort(g, kind="stable")
    gs = g[order]
    cnt = np.bincount(g, minlength=N_BUCKETS)
    if cnt.max() > CAP:
        raise ValueError(f"bucket overflow: {cnt.max()} > {CAP}")
    bounds = np.zeros(N_BUCKETS + 1, np.int64)
    bounds[1:] = np.cumsum(cnt)

    local_sorted = (idx64[order] & (WINDOW - 1)).astype(np.int16)
    padded = np.zeros((N_BUCKETS, CAP), np.int16)
    for gb in range(N_BUCKETS):
        seg = local_sorted[bounds[gb]:bounds[gb + 1]]
        padded[gb, :len(seg)] = seg

    tiles = padded.reshape(N_BUCKETS, CHUNKS, IDX_COLS, 16)
    tiles = tiles.transpose(0, 1, 3, 2)              # [gb, t, 16, IDX_COLS]
    tiles = np.tile(tiles, (1, 1, 8, 1))             # replicate across Q7 cores
    per_core_idx = np.ascontiguousarray(
        tiles.reshape(N_CORES, N_GATHERS, P, IDX_COLS)
    )

    # gathered position k (sorted order) -> row in the concatenated output
    w = np.arange(N_IDX, dtype=np.int64) - bounds[gs]
    c = gs >> 2
    b = gs & 3
    t = w // CHUNK_IDX
    i = w % CHUNK_IDX
    rows = (c * OUT_PER_CORE + (b * CHUNKS + t) * CHUNK_IDX
            + (i % P) * NBLK + i // P)
    return per_core_idx, order, rows


def _run(weight, index, trace=False):
    from concourse import bass_utils

    if trace:
        _ensure_ntff_hook()
    nc = _build_nc()

    wpad = np.zeros((N_EMB_PAD, D), np.float32)
    wpad[:N_EMB] = np.asarray(weight, dtype=np.float32)
    wshards = wpad.reshape(N_CORES, SHARD_ROWS, D)

    per_core_idx, order, rows = _route(index)

    in_maps = [{"wshard": wshards[ci], "idx": per_core_idx[ci]}
               for ci in range(N_CORES)]
    res = bass_utils.run_bass_kernel_spmd(
        nc, in_maps, core_ids=list(range(N_CORES)), trace=trace
    )
    gathered = np.concatenate(
        [res.results[ci]["out"] for ci in range(N_CORES)], axis=0
    )
    full = np.empty((N_IDX, D), np.float32)
    full[order] = gathered[rows]
    return full, res


def kernel(weight, index):
    full, _ = _run(weight, index, trace=False)
    return full


# revision 6
# speedup vs baseline: 1.0872x; 1.0872x over previous
"""Embedding lookup (nn.Embedding forward) on 8 TRN2 NeuronCores.

Strategy (per the row-sharding hint, with the index routing done host-side):
the 1M x 128 fp32 table is row-sharded into 8 contiguous shards of 131072
rows (table padded to 1,048,576 rows), one per core -- 64 MB each.  The host
routes each of the 2,097,152 indices to the owning core, and within a core to
one of four 32768-row windows, so the on-device gather can use the bulk
`dma_gather` instruction (int16 local indices, one 512 B descriptor per row,
descriptor generation spread across the 8 GpSimd Q7 cores).  Each (core,
window) bucket is padded to a fixed capacity so all 8 cores run the same SPMD
program; the host applies the inverse permutation to the concatenated per-core
outputs to restore the original index order.

Per-core HW traffic: ~147 MB gather reads + ~147 MB output writes at the
~360 GB/s per-core HBM limit -> ~820 us roofline.
"""

import sys

if "/opt/trn_rl_repo" not in sys.path:
    sys.path.insert(0, "/opt/trn_rl_repo")

import numpy as np

N_CORES = 8
N_EMB = 1_000_000
D = 128
N_IDX = 2_097_152
P = 128

WINDOW = 32768                     # rows addressable by one int16 gather
BUCKETS_PER_CORE = 4
SHARD_ROWS = WINDOW * BUCKETS_PER_CORE      # 131072
N_EMB_PAD = SHARD_ROWS * N_CORES            # 1048576
N_BUCKETS = N_CORES * BUCKETS_PER_CORE      # 32

CHUNK_IDX = 11776                  # indices per dma_gather (nblk = 92)
NBLK = CHUNK_IDX // P              # 92
CHUNKS = 6                         # chunks per bucket
CAP = CHUNK_IDX * CHUNKS           # 70656 padded capacity per bucket
N_GATHERS = BUCKETS_PER_CORE * CHUNKS       # 20 per core
OUT_PER_CORE = CAP * BUCKETS_PER_CORE       # 286720 rows
IDX_COLS = CHUNK_IDX // 16         # 896 int16 per partition per chunk

_NC_CACHE = None


def _build_nc():
    global _NC_CACHE
    if _NC_CACHE is not None:
        return _NC_CACHE

    from concourse import bacc, mybir, tile

    nc = bacc.Bacc("TRN2", target_bir_lowering=False, debug=False,
                   num_devices=N_CORES)
    w = nc.dram_tensor("wshard", (SHARD_ROWS, D), mybir.dt.float32,
                       kind="ExternalInput")
    idxt = nc.dram_tensor("idx", (N_GATHERS, P, IDX_COLS), mybir.dt.int16,
                          kind="ExternalInput")
    out = nc.dram_tensor("out", (OUT_PER_CORE, D), mybir.dt.float32,
                         kind="ExternalOutput")

    with tile.TileContext(nc) as tc:
        with tc.tile_pool(name="ip", bufs=N_GATHERS) as ip, \
             tc.tile_pool(name="gp", bufs=3) as gp:
            # Preload every index tile (35 KB total) so the POOL engine's
            # descriptor-generation stream never stalls on an index DMA.
            idx_tiles = []
            for k in range(N_GATHERS):
                it = ip.tile([P, IDX_COLS], mybir.dt.int16)
                nc.sync.dma_start(it[:], idxt[k, :, :])
                idx_tiles.append(it)
            for b in range(BUCKETS_PER_CORE):
                win = w[b * WINDOW:(b + 1) * WINDOW, :]
                for t in range(CHUNKS):
                    k = b * CHUNKS + t
                    g = gp.tile([P, NBLK * D], mybir.dt.float32)
                    nc.gpsimd.dma_gather(
                        out_ap=g[:].rearrange("p (n d) -> p n d", d=D),
                        in_ap=win,
                        idxs_ap=idx_tiles[k][:],
                        num_idxs=CHUNK_IDX,
                        num_idxs_reg=CHUNK_IDX,
                        elem_size=D,
                        single_packet=False,
                    )
                    # DRAM row k*CHUNK_IDX + p*NBLK + j  <-  tile[p, j]
                    # Stores ride the scalar (ACT) HWDGE ring so they don't
                    # queue behind the sync-ring index loads.
                    dst = out[k * CHUNK_IDX:(k + 1) * CHUNK_IDX, :]
                    nc.scalar.dma_start(
                        dst.rearrange("(p n) d -> p n d", p=P), g[:]
                    )

    nc.compile()
    _NC_CACHE = nc
    return nc


def _ensure_ntff_hook():
    """The agent image's antenv lacks axon_hooks, so run_bass_kernel_spmd's
    trace path can't find the NTFF profile hook trn_boot builds.  Shim the
    module and install the ctypes hook ourselves; also neuter the bucket
    upload (no artifact store in this container)."""
    import sys as _sys
    import types

    if "antenv.axon_hooks" not in _sys.modules:
        mod = types.ModuleType("antenv.axon_hooks")
        mod._hook = None

        def set_axon_ntff_profile_hook(h):
            mod._hook = h

        def get_axon_ntff_profile_hook():
            return mod._hook

        mod.set_axon_ntff_profile_hook = set_axon_ntff_profile_hook
        mod.get_axon_ntff_profile_hook = get_axon_ntff_profile_hook
        _sys.modules["antenv.axon_hooks"] = mod
        import antenv

        antenv.axon_hooks = mod

    from antenv.axon_hooks import (get_axon_ntff_profile_hook,
                                   set_axon_ntff_profile_hook)

    if get_axon_ntff_profile_hook() is None:
        from trn_agent_boot.trn_boot import _ntff_profile_via_ctypes

        set_axon_ntff_profile_hook(
            _ntff_profile_via_ctypes("/opt/axon/libaxon_pjrt.so")
        )

    from concourse import bass_utils

    bass_utils.upload_artifacts = lambda tmpdir: f"local://{tmpdir}"


def _route(index):
    """Host-side routing: bucket each index by value, pad buckets to CAP,
    build the per-core int16 gather-index tiles and the gather->original
    permutation."""
    idx64 = np.asarray(index).astype(np.int64)
    g = idx64 >> 15                                  # owning bucket, 0..30
    order = np.argsort(g, kind="stable")
    gs = g[order]
    cnt = np.bincount(g, minlength=N_BUCKETS)
    if cnt.max() > CAP:
        raise ValueError(f"bucket overflow: {cnt.max()} > {CAP}")
    bounds = np.zeros(N_BUCKETS + 1, np.int64)
    bounds[1:] = np.cumsum(cnt)

    local_sorted = (idx64[order] & (WINDOW - 1)).astype(np.int16)
    padded = np.zeros((N_BUCKETS, CAP), np.int16)
    for gb in range(N_BUCKETS):
        seg = local_sorted[bounds[gb]:bounds[gb + 1]]
        padded[gb, :len(seg)] = seg

    tiles = padded.reshape(N_BUCKETS, CHUNKS, IDX_COLS, 16)
    tiles = tiles.transpose(0, 1, 3, 2)              # [gb, t, 16, IDX_COLS]
    tiles = np.tile(tiles, (1, 1, 8, 1))             # replicate across Q7 cores
    per_core_idx = np.ascontiguousarray(
        tiles.reshape(N_CORES, N_GATHERS, P, IDX_COLS)
    )

    # gathered position k (sorted order) -> row in the concatenated output
    w = np.arange(N_IDX, dtype=np.int64) - bounds[gs]
    c = gs >> 2
    b = gs & 3
    t = w // CHUNK_IDX
    i = w % CHUNK_IDX
    rows = (c * OUT_PER_CORE + (b * CHUNKS + t) * CHUNK_IDX
            + (i % P) * NBLK + i // P)
    return per_core_idx, order, rows


def _run(weight, index, trace=False):
    from concourse import bass_utils

    if trace:
        _ensure_ntff_hook()
    nc = _build_nc()

    wpad = np.zeros((N_EMB_PAD, D), np.float32)
    wpad[:N_EMB] = np.asarray(weight, dtype=np.float32)
    wshards = wpad.reshape(N_CORES, SHARD_ROWS, D)

    per_core_idx, order, rows = _route(index)

    in_maps = [{"wshard": wshards[ci], "idx": per_core_idx[ci]}
               for ci in range(N_CORES)]
    res = bass_utils.run_bass_kernel_spmd(
        nc, in_maps, core_ids=list(range(N_CORES)), trace=trace
    )
    gathered = np.concatenate(
        [res.results[ci]["out"] for ci in range(N_CORES)], axis=0
    )
    full = np.empty((N_IDX, D), np.float32)
    full[order] = gathered[rows]
    return full, res


def kernel(weight, index):
    full, _ = _run(weight, index, trace=False)
    return full
